# revision 1
# baseline (speedup 1.0000x reference)
"""Trainium2 Bass kernel for nn_CrossAttention_65051574665735.

Cross-attention block (MQA, shared K/V head) + parallel SwiGLU FF.
Data-parallel over B*N rows across 8 NeuronCores: core c handles batch c//4,
rows (c%4)*512. Context + weights replicated (weights pre-cast to bf16/fp8
with the layernorm scale g and the 1/sqrt(dh) attention scale folded in on
the host). No cross-core collectives; the host concatenates the 8 slices.

Schedule notes:
- All layernorm transposes go through the DMA crossbar (dma_start_transpose),
  keeping PE for matmuls and DVE for element-wise work.
- The scalar (ACT) engine runs a single activation function per phase to
  avoid act-table reloads: Sqrt during layernorms, Exp during attention,
  Silu during the FF down-projection.
- FF1 up-proj matmuls are emitted in small "steps" interleaved into the ctx
  and attention phases to fill PE gaps; the SwiGLU nonlinearity is deferred
  to phase E (raw a/gate stored in SBUF as bf16).
- Q projection uses fp8 DoubleRow (weights pre-scaled x256 on the host,
  un-scaled on the PSUM->SBUF copy).
- Phase E accumulates Wo + FF2 into PSUM split by output column half so the
  first half drains (copy + store) while the second half computes.
"""

import sys

if "/opt/trn_rl_repo" not in sys.path:
    sys.path.insert(0, "/opt/trn_rl_repo")

import numpy as np
import ml_dtypes

import concourse.bass as bass
import concourse.tile as tile
from concourse import mybir, bacc
from concourse.masks import make_identity

F32 = mybir.dt.float32
BF16 = mybir.dt.bfloat16
FP8 = mybir.dt.float8e4

B, N, J = 2, 2048, 2048
DIM, HEADS, DH = 1024, 16, 64
INNER = HEADS * DH
FF = 4 * DIM
EPS = 1e-5
N_CORES = 8
R = B * N // N_CORES  # 512 rows per core
KT = DIM // 128  # 8 contraction tiles over dim
KP = KT // 2  # 4 fp8 contraction pairs
RT = R // 128  # 4 row tiles
CT = J // 128  # 16 context row tiles
FT = FF // 128  # 32 ff tiles
QSC = 256.0  # host pre-scale on Wq for fp8


def _ln_tile(nc, pools, src_dram, t, bias_tile):
    """LN one 128-row tile of src_dram; returns normalized [128, DIM] bf16."""
    ln_pool, stats_pool, eps_tile = pools
    x_t = ln_pool.tile([128, DIM], F32, tag="ln_x")
    nc.gpsimd.dma_start(x_t[:], src_dram[t * 128 : (t + 1) * 128, :])
    stats = stats_pool.tile([128, 2, nc.vector.BN_STATS_DIM], F32, tag="st")
    nc.vector.bn_stats(stats[:, 0, :], x_t[:, 0:512])
    nc.vector.bn_stats(stats[:, 1, :], x_t[:, 512:1024])
    mv = stats_pool.tile([128, nc.vector.BN_AGGR_DIM], F32, tag="mv")
    nc.vector.bn_aggr(mv[:], stats[:])
    rstd = stats_pool.tile([128, 1], F32, tag="rs")
    nc.scalar.activation(
        rstd[:], mv[:, 1:2], mybir.ActivationFunctionType.Sqrt, bias=eps_tile[:]
    )
    nc.vector.reciprocal(rstd[:], rstd[:])
    xn_t = ln_pool.tile([128, DIM], BF16, tag="ln_xn")
    nc.vector.tensor_scalar(
        out=xn_t[:],
        in0=x_t[:],
        scalar1=mv[:, 0:1],
        scalar2=rstd[:],
        op0=mybir.AluOpType.subtract,
        op1=mybir.AluOpType.mult,
    )
    if bias_tile is not None:
        nc.vector.tensor_add(xn_t[:], xn_t[:], bias_tile[:])
    return xn_t


def build_kernel(x_bias_nonzero: bool, c_bias_nonzero: bool):
    nc = bacc.Bacc(
        "TRN2", target_bir_lowering=False, debug=False, num_devices=N_CORES
    )
    d_x = nc.dram_tensor("x", [R, DIM], F32, kind="ExternalInput").ap()
    d_ctx = nc.dram_tensor("ctx", [J, DIM], F32, kind="ExternalInput").ap()
    d_wq8 = nc.dram_tensor("wq8", [DIM, INNER], FP8, kind="ExternalInput").ap()
    d_wkv = nc.dram_tensor("wkv", [DIM, 2 * DH], BF16, kind="ExternalInput").ap()
    d_wo = nc.dram_tensor("wo", [INNER, DIM], BF16, kind="ExternalInput").ap()
    d_wff1 = nc.dram_tensor("wff1", [DIM, 2 * FF], BF16, kind="ExternalInput").ap()
    d_wff2 = nc.dram_tensor("wff2", [FF, DIM], BF16, kind="ExternalInput").ap()
    d_xb = (
        nc.dram_tensor("xb", [1, DIM], F32, kind="ExternalInput").ap()
        if x_bias_nonzero
        else None
    )
    d_cb = (
        nc.dram_tensor("cb", [1, DIM], F32, kind="ExternalInput").ap()
        if c_bias_nonzero
        else None
    )
    d_out = nc.dram_tensor("out", [R, DIM], F32, kind="ExternalOutput").ap()

    with tile.TileContext(nc) as tc:
        with (
            tc.tile_pool(name="consts", bufs=1) as consts,
            tc.tile_pool(name="persist", bufs=1) as persist,
            tc.tile_pool(name="wo", bufs=1) as wo_pool,
        ):
            ident = consts.tile([128, 128], BF16)
            make_identity(nc, ident)
            eps_tile = consts.tile([128, 1], F32, tag="eps")
            nc.vector.memset(eps_tile[:], EPS)

            xb_tile = cb_tile = None
            if d_xb is not None:
                xb_tile = consts.tile([128, DIM], F32, tag="xb")
                nc.gpsimd.dma_start(
                    xb_tile[:],
                    bass.AP(
                        tensor=d_xb.tensor, offset=d_xb.offset,
                        ap=[[0, 128]] + d_xb.ap[1:],
                    ),
                )
            if d_cb is not None:
                cb_tile = consts.tile([128, DIM], F32, tag="cb")
                nc.gpsimd.dma_start(
                    cb_tile[:],
                    bass.AP(
                        tensor=d_cb.tensor, offset=d_cb.offset,
                        ap=[[0, 128]] + d_cb.ap[1:],
                    ),
                )

            xnT = persist.tile([128, KT, R], BF16, tag="xnT")
            xnT8 = [
                persist.tile([128, 2, R], FP8, tag=f"xnT8{p}", name=f"xnT8{p}")
                for p in range(KP)
            ]
            kT = persist.tile([128, J], BF16, tag="kT")
            vo = [
                persist.tile([128, DH + 1], BF16, tag=f"vo{j}", name=f"vo{j}")
                for j in range(CT)
            ]
            aoT = [
                persist.tile([128, R], BF16, tag=f"aoT{k}", name=f"aoT{k}")
                for k in range(KT)
            ]
            qT = [
                persist.tile([128, R], BF16, tag=f"qT{h}", name=f"qT{h}")
                for h in range(HEADS // 2)
            ]
            aT = [
                persist.tile([128, R], BF16, tag=f"aT{f}", name=f"aT{f}")
                for f in range(FT)
            ]
            gT = [
                persist.tile([128, R], BF16, tag=f"gT{f}", name=f"gT{f}")
                for f in range(FT)
            ]
            wo_sb = [
                wo_pool.tile([128, DIM], BF16, tag=f"wo{k}", name=f"wo{k}")
                for k in range(KT)
            ]

            with tc.tile_pool(name="wff1", bufs=16) as wff1_pool:
                # ---- FF1 unit generator (paced into phases C and D) ----
                w1_tiles = {}

                def ff1_steps(fi):
                    g = fi // 8
                    fl = fi % 8
                    if fl == 0:
                        w1a = []
                        w1g = []
                        for k in range(KT):
                            ta = wff1_pool.tile([128, 1024], BF16, tag="w1")
                            nc.sync.dma_start(
                                ta[:],
                                d_wff1[
                                    k * 128 : (k + 1) * 128,
                                    g * 1024 : (g + 1) * 1024,
                                ],
                            )
                            w1a.append(ta)
                            tg = wff1_pool.tile([128, 1024], BF16, tag="w1")
                            nc.sync.dma_start(
                                tg[:],
                                d_wff1[
                                    k * 128 : (k + 1) * 128,
                                    FF + g * 1024 : FF + (g + 1) * 1024,
                                ],
                            )
                            w1g.append(tg)
                        w1_tiles[g] = (w1a, w1g)
                    w1a, w1g = w1_tiles[g]
                    a_ps = psum_f.tile([128, R], F32, tag="ffa")
                    g_ps = psum_f.tile([128, R], F32, tag="ffg")

                    def mk_chain(ps, w1, k0):
                        def emit():
                            for k in range(k0, k0 + 4):
                                nc.tensor.matmul(
                                    ps[:],
                                    w1[k][:, fl * 128 : (fl + 1) * 128],
                                    xnT[:, k, :],
                                    start=(k == 0),
                                    stop=(k == KT - 1),
                                )
                        return emit

                    def finish():
                        for k in range(4, 8):
                            nc.tensor.matmul(
                                g_ps[:],
                                w1g[k][:, fl * 128 : (fl + 1) * 128],
                                xnT[:, k, :],
                                start=False,
                                stop=(k == KT - 1),
                            )
                        nc.vector.tensor_copy(aT[fi][:], a_ps[:])
                        nc.vector.tensor_copy(gT[fi][:], g_ps[:])

                    return [
                        mk_chain(a_ps, w1a, 0),
                        mk_chain(a_ps, w1a, 4),
                        mk_chain(g_ps, w1g, 0),
                        finish,
                    ]

                ff_queue = []
                ff_next = [0]

                def ff_step(n=1):
                    for _ in range(n):
                        if not ff_queue and ff_next[0] < FT:
                            ff_queue.extend(ff1_steps(ff_next[0]))
                            ff_next[0] += 1
                        if ff_queue:
                            ff_queue.pop(0)()

                with (
                    tc.tile_pool(name="ln", bufs=2) as ln_pool,
                    tc.tile_pool(name="stats", bufs=3) as stats_pool,
                    tc.tile_pool(name="wq8", bufs=1) as wq8_pool,
                    tc.tile_pool(name="cnT", bufs=1) as cnT_pool,
                    tc.tile_pool(name="wkv", bufs=1) as wkv_pool,
                    tc.tile_pool(name="vstage", bufs=2) as vstage,
                    tc.tile_pool(name="psA", bufs=2, space="PSUM") as psum_tr,
                    tc.tile_pool(name="psF", bufs=1, space="PSUM") as psum_f,
                    tc.tile_pool(name="psKV", bufs=2, space="PSUM") as psum_kv,
                    tc.tile_pool(name="psQ", bufs=2, space="PSUM") as psum_q,
                ):
                    ln_pools = (ln_pool, stats_pool, eps_tile)
                    wq8_sb = [
                        wq8_pool.tile(
                            [128, 2, INNER], FP8, tag=f"wq{p}", name=f"wq{p}"
                        )
                        for p in range(KP)
                    ]
                    for p in range(KP):
                        for i in range(2):
                            nc.sync.dma_start(
                                wq8_sb[p][:, i, :],
                                d_wq8[(2 * p + i) * 128 : (2 * p + i + 1) * 128, :],
                            )
                    wkv_sb = [
                        wkv_pool.tile(
                            [128, 2 * DH], BF16, tag=f"wkv{k}", name=f"wkv{k}"
                        )
                        for k in range(KT)
                    ]
                    for k in range(KT):
                        nc.sync.dma_start(
                            wkv_sb[k][:], d_wkv[k * 128 : (k + 1) * 128, :]
                        )

                    # ---- Phase A: x layernorm + DMA transposes ----
                    for t in range(RT):
                        xn_t = _ln_tile(nc, ln_pools, d_x, t, xb_tile)
                        nc.scalar.dma_start_transpose(
                            xnT[:, :, t * 128 : (t + 1) * 128], xn_t[:]
                        )
                    for p in range(KP):
                        for i in range(2):
                            nc.vector.tensor_copy(
                                xnT8[p][:, i, :], xnT[:, 2 * p + i, :]
                            )

                    # ---- Phase B: Q projection (fp8 DoubleRow) ----
                    for hp in range(HEADS // 2):
                        q_ps = psum_q.tile([128, R], F32, tag="q")
                        for p in range(KP):
                            nc.tensor.matmul(
                                q_ps[:],
                                wq8_sb[p][:, :, hp * 128 : (hp + 1) * 128],
                                xnT8[p][:],
                                start=(p == 0),
                                stop=(p == KP - 1),
                                perf_mode=mybir.MatmulPerfMode.DoubleRow,
                            )
                        nc.vector.tensor_scalar(
                            out=qT[hp][:],
                            in0=q_ps[:],
                            scalar1=1.0 / QSC,
                            scalar2=None,
                            op0=mybir.AluOpType.mult,
                        )

                    # ---- Phase C: ctx layernorm + KV projection ----
                    cnT = cnT_pool.tile([128, KT, J], BF16, tag="cnT")
                    for c in range(J // 512):
                        for t4 in range(4):
                            t = c * 4 + t4
                            cn_t = _ln_tile(nc, ln_pools, d_ctx, t, cb_tile)
                            nc.scalar.dma_start_transpose(
                                cnT[:, :, t * 128 : (t + 1) * 128], cn_t[:]
                            )
                        kv_ps = psum_kv.tile([128, 512], F32, tag="kv")
                        for k in range(KT):
                            nc.tensor.matmul(
                                kv_ps[:],
                                wkv_sb[k][:],
                                cnT[:, k, c * 512 : (c + 1) * 512],
                                start=(k == 0),
                                stop=(k == KT - 1),
                            )
                        nc.vector.tensor_copy(
                            kT[0:DH, c * 512 : (c + 1) * 512], kv_ps[0:DH, :]
                        )
                        nc.gpsimd.dma_start(
                            kT[DH:128, c * 512 : (c + 1) * 512],
                            kT[0:DH, c * 512 : (c + 1) * 512],
                        )
                        vT_sb = vstage.tile([128, 512], BF16, tag="vT")
                        nc.vector.tensor_copy(vT_sb[DH:128, :], kv_ps[DH:128, :])
                        for j4 in range(4):
                            jc = c * 4 + j4
                            vps = psum_tr.tile([128, DH], BF16, tag="tr")
                            nc.tensor.transpose(
                                vps[:],
                                vT_sb[DH:128, j4 * 128 : (j4 + 1) * 128],
                                ident[DH:128, DH:128],
                            )
                            nc.vector.tensor_copy(vo[jc][:, 0:DH], vps[:])
                            nc.vector.memset(vo[jc][:, DH : DH + 1], 1.0)
                        if c >= 1:
                            ff_step(4)

                # ---- Phase D: attention, FF1 interleaved ----
                with (
                    tc.tile_pool(name="attn", bufs=9) as attn_pool,
                    tc.tile_pool(name="smx", bufs=4) as smx_pool,
                    tc.tile_pool(name="psS", bufs=2, space="PSUM") as psum_s,
                    tc.tile_pool(name="psAV", bufs=2, space="PSUM") as psum_av,
                    tc.tile_pool(name="psF2", bufs=1, space="PSUM") as psum_f2,
                ):
                    psum_f = psum_f2
                    for hp in range(HEADS // 2):
                        av_ps = [None, None]
                        for h2 in range(2):
                            qh = qT[hp][h2 * 64 : (h2 + 1) * 64, :]
                            a_sbs = []
                            for p in range(CT // 2):
                                s_ps = psum_s.tile([128, 2 * R], F32, tag="sim")
                                for half in range(2):
                                    jc = 2 * p + half
                                    nc.tensor.matmul(
                                        s_ps[:, half * R : (half + 1) * R],
                                        kT[
                                            h2 * DH : (h2 + 1) * DH,
                                            jc * 128 : (jc + 1) * 128,
                                        ],
                                        qh,
                                        start=True,
                                        stop=True,
                                    )
                                a_sb = attn_pool.tile([128, 2 * R], BF16, tag="at")
                                nc.scalar.activation(
                                    a_sb[:],
                                    s_ps[:],
                                    mybir.ActivationFunctionType.Exp,
                                )
                                a_sbs.append(a_sb)
                                ff_step(1)
                            av = psum_av.tile([DH + 1, R], F32, tag="av")
                            av_ps[h2] = av
                            for jc in range(CT):
                                nc.tensor.matmul(
                                    av[:],
                                    vo[jc][:],
                                    a_sbs[jc // 2][
                                        :, (jc % 2) * R : (jc % 2 + 1) * R
                                    ],
                                    start=(jc == 0),
                                    stop=(jc == CT - 1),
                                )
                                if jc in (5, 11):
                                    ff_step(1)
                        for h2 in range(2):
                            av = av_ps[h2]
                            rec = smx_pool.tile([DH + 1, R], F32, tag="rec")
                            nc.vector.reciprocal(
                                rec[DH : DH + 1, :], av[DH : DH + 1, :]
                            )
                            rec0 = smx_pool.tile([1, R], F32, tag="rec0")
                            nc.gpsimd.dma_start(rec0[:], rec[DH : DH + 1, :])
                            rbc = smx_pool.tile([DH, R], F32, tag="rbc")
                            nc.gpsimd.partition_broadcast(rbc[:], rec0[:])
                            if h2 == 0:
                                nc.vector.tensor_mul(
                                    aoT[hp][0:DH, :], av[0:DH, :], rbc[:]
                                )
                            else:
                                tmp = smx_pool.tile([DH, R], BF16, tag="aotmp")
                                nc.vector.tensor_mul(tmp[:], av[0:DH, :], rbc[:])
                                nc.gpsimd.dma_start(aoT[hp][DH:128, :], tmp[:])
                        ff_step(1)
                        if hp == 5:
                            for k in range(KT):
                                nc.sync.dma_start(
                                    wo_sb[k][:], d_wo[k * 128 : (k + 1) * 128, :]
                                )
                    while ff_queue or ff_next[0] < FT:
                        ff_step(1)

            # ---- Phase E: SwiGLU + Wo + FF2, split by output column half ----
            with (
                tc.tile_pool(name="wff2", bufs=8) as wff2_pool,
                tc.tile_pool(name="hpool", bufs=1) as hpool,
                tc.tile_pool(name="hstage", bufs=3) as hstage,
                tc.tile_pool(name="ostage", bufs=4) as ostage,
                tc.tile_pool(name="psO", bufs=1, space="PSUM") as psum_o,
            ):
                hT = {}
                for ch in range(2):
                    o_ps = [
                        psum_o.tile(
                            [128, 512], F32, tag=f"o{ch}{rs}", name=f"o{ch}{rs}"
                        )
                        for rs in range(RT)
                    ]
                    for k in range(KT):
                        for rs in range(RT):
                            nc.tensor.matmul(
                                o_ps[rs][:],
                                aoT[k][:, rs * 128 : (rs + 1) * 128],
                                wo_sb[k][:, ch * 512 : (ch + 1) * 512],
                                start=(k == 0),
                                stop=False,
                            )
                    for fi in range(FT):
                        w2 = wff2_pool.tile([128, 512], BF16, tag="w2")
                        nc.sync.dma_start(
                            w2[:],
                            d_wff2[
                                fi * 128 : (fi + 1) * 128,
                                ch * 512 : (ch + 1) * 512,
                            ],
                        )
                        if ch == 0:
                            sil = hstage.tile([128, R], F32, tag="sil")
                            nc.scalar.activation(
                                sil[:],
                                gT[fi][:],
                                mybir.ActivationFunctionType.Silu,
                            )
                            h = hT[fi] = hpool.tile(
                                [128, R], BF16, tag=f"h{fi}", name=f"h{fi}"
                            )
                            nc.vector.tensor_mul(h[:], aT[fi][:], sil[:])
                        for rs in range(RT):
                            nc.tensor.matmul(
                                o_ps[rs][:],
                                hT[fi][:, rs * 128 : (rs + 1) * 128],
                                w2[:],
                                start=False,
                                stop=(fi == FT - 1),
                            )
                    for rs in range(RT):
                        o_sb = ostage.tile([128, 512], F32, tag="ost")
                        nc.vector.tensor_copy(o_sb[:], o_ps[rs][:])
                        nc.gpsimd.dma_start(
                            d_out[
                                rs * 128 : (rs + 1) * 128,
                                ch * 512 : (ch + 1) * 512,
                            ],
                            o_sb[:],
                        )

    nc.compile()
    return nc


_NC_CACHE = {}


def _get_nc(x_bias_nonzero, c_bias_nonzero):
    key = (x_bias_nonzero, c_bias_nonzero)
    if key not in _NC_CACHE:
        _NC_CACHE[key] = build_kernel(*key)
    return _NC_CACHE[key]


def make_in_maps(x, context, norm_g, norm_b, cnorm_g, cnorm_b, Wq, Wkv, Wo, Wff1, Wff2):
    x = np.asarray(x, np.float32)
    context = np.asarray(context, np.float32)
    norm_g = np.asarray(norm_g, np.float32)
    norm_b = np.asarray(norm_b, np.float32)
    cnorm_g = np.asarray(cnorm_g, np.float32)
    cnorm_b = np.asarray(cnorm_b, np.float32)
    scale = DH ** -0.5
    bf = ml_dtypes.bfloat16
    f8 = ml_dtypes.float8_e4m3
    wq8 = np.ascontiguousarray(
        np.clip(
            norm_g[:, None] * np.asarray(Wq, np.float32) * scale * QSC, -240, 240
        )
    ).astype(f8)
    wkv = np.ascontiguousarray(cnorm_g[:, None] * np.asarray(Wkv, np.float32)).astype(bf)
    wo = np.ascontiguousarray(np.asarray(Wo, np.float32)).astype(bf)
    wff1 = np.ascontiguousarray(norm_g[:, None] * np.asarray(Wff1, np.float32)).astype(bf)
    wff2 = np.ascontiguousarray(np.asarray(Wff2, np.float32)).astype(bf)
    x_bias = bool(np.any(norm_b != 0.0))
    c_bias = bool(np.any(cnorm_b != 0.0))
    in_maps = []
    for c in range(N_CORES):
        b = c // (N_CORES // B)
        r0 = (c % (N_CORES // B)) * R
        m = {
            "x": np.ascontiguousarray(x[b, r0 : r0 + R, :]),
            "ctx": np.ascontiguousarray(context[b]),
            "wq8": wq8,
            "wkv": wkv,
            "wo": wo,
            "wff1": wff1,
            "wff2": wff2,
        }
        if x_bias:
            m["xb"] = norm_b.reshape(1, DIM).copy()
        if c_bias:
            m["cb"] = cnorm_b.reshape(1, DIM).copy()
        in_maps.append(m)
    return in_maps, x_bias, c_bias


def gather_output(results):
    out = np.empty((B, N, DIM), np.float32)
    for c in range(N_CORES):
        b = c // (N_CORES // B)
        r0 = (c % (N_CORES // B)) * R
        out[b, r0 : r0 + R, :] = results[c]["out"]
    return out


def kernel(**inputs):
    from concourse.bass_utils import run_bass_kernel_spmd

    in_maps, x_bias, c_bias = make_in_maps(**inputs)
    nc = _get_nc(x_bias, c_bias)
    res = run_bass_kernel_spmd(nc, in_maps, list(range(N_CORES)))
    return gather_output(res.results)



# revision 7
# speedup vs baseline: 1.0168x; 1.0168x over previous
"""Trainium2 Bass kernel for nn_CrossAttention_65051574665735.

Cross-attention block (MQA, shared K/V head) + parallel SwiGLU FF.
Data-parallel over B*N rows across 8 NeuronCores: core c handles batch c//4,
rows (c%4)*512. Context + weights replicated (weights pre-cast to bf16/fp8
with the layernorm scale g and the 1/sqrt(dh) attention scale folded in on
the host). No cross-core collectives; the host concatenates the 8 slices.

Schedule notes (v2):
- Weight/x/ctx DMAs are batched into a handful of large multi-dim
  descriptors (sync queue for weights + ctx transposes, gpsimd for x/ctx
  loads and stores) so queue-issue time stops being a bottleneck.
- x layernorm transposes run on the (otherwise idle) PE via identity
  matmuls; LN normalize runs on ACT (Identity with per-row scale+bias)
  so DVE only does stats + copies.
- sim matmuls for the two heads of a pair are issued back-to-back into
  disjoint PE row groups (K=64 each) so they execute concurrently.
- softmax probs are written as fp8e4 directly by the ACT exp; AV runs as
  fp8 DoubleRow over j-tile pairs (vo pair tiles [128,2,80], ones column
  at col 64 gives the softmax denominator for free).
- FF1 up-proj matmuls are emitted in small "steps" interleaved into the
  ctx and attention phases to fill PE gaps; SwiGLU (silu*a) is deferred
  into phase E where ACT is free, h overwrites gT in place.
- Phase E streams Wo + FF2 with N=1024 rhs into 4 row-tile PSUM banks and
  stores results straight from PSUM to DRAM.
"""

import sys

if "/opt/trn_rl_repo" not in sys.path:
    sys.path.insert(0, "/opt/trn_rl_repo")

import numpy as np
import ml_dtypes

import concourse.bass as bass
import concourse.tile as tile
from concourse import mybir, bacc
from concourse.masks import make_identity

F32 = mybir.dt.float32
BF16 = mybir.dt.bfloat16
FP8 = mybir.dt.float8e4

B, N, J = 2, 2048, 2048
DIM, HEADS, DH = 1024, 16, 64
INNER = HEADS * DH
FF = 4 * DIM
EPS = 1e-5
N_CORES = 8
R = B * N // N_CORES  # 512 rows per core
KT = DIM // 128  # 8 contraction tiles over dim
KP = KT // 2  # 4 fp8 contraction pairs
RT = R // 128  # 4 row tiles
CT = J // 128  # 16 context row tiles
FT = FF // 128  # 32 ff tiles
QSC = 256.0  # host pre-scale on Wq for fp8
VOW = 80  # vo pair tile free width (65 used; 80 for DR step%16==0)


def _ap3(d, n0, n1, n2, off=0):
    """3D view [n0 part, n1, n2] over a 2D dram tensor d with row length n2."""
    return bass.AP(
        tensor=d.tensor,
        offset=d.offset + off,
        ap=[[n2, n0], [n0 * n2, n1], [1, n2]],
    )


def build_kernel(x_bias_nonzero: bool, c_bias_nonzero: bool):
    nc = bacc.Bacc(
        "TRN2", target_bir_lowering=False, debug=False, num_devices=N_CORES
    )
    d_x = nc.dram_tensor("x", [R, DIM], F32, kind="ExternalInput").ap()
    d_ctx = nc.dram_tensor("ctx", [J, DIM], F32, kind="ExternalInput").ap()
    d_wq8 = nc.dram_tensor("wq8", [DIM, INNER], FP8, kind="ExternalInput").ap()
    d_wkv = nc.dram_tensor("wkv", [DIM, 2 * DH], BF16, kind="ExternalInput").ap()
    d_wo = nc.dram_tensor("wo", [INNER, DIM], BF16, kind="ExternalInput").ap()
    d_wff1 = nc.dram_tensor("wff1", [DIM, 2 * FF], BF16, kind="ExternalInput").ap()
    d_wff2 = nc.dram_tensor("wff2", [FF, DIM], BF16, kind="ExternalInput").ap()
    d_xb = (
        nc.dram_tensor("xb", [1, DIM], F32, kind="ExternalInput").ap()
        if x_bias_nonzero
        else None
    )
    d_cb = (
        nc.dram_tensor("cb", [1, DIM], F32, kind="ExternalInput").ap()
        if c_bias_nonzero
        else None
    )
    d_out = nc.dram_tensor("out", [R, DIM], F32, kind="ExternalOutput").ap()

    with tile.TileContext(nc) as tc:
        with (
            tc.tile_pool(name="consts", bufs=1) as consts,
            tc.tile_pool(name="persist", bufs=1) as persist,
            tc.tile_pool(name="wo", bufs=1) as wo_pool,
        ):
            ident = consts.tile([128, 128], BF16)
            make_identity(nc, ident)
            eps_tile = consts.tile([128, 1], F32, tag="eps")
            nc.vector.memset(eps_tile[:], EPS)

            xb_tile = cb_tile = None
            if d_xb is not None:
                xb_tile = consts.tile([128, DIM], F32, tag="xb")
                nc.gpsimd.dma_start(
                    xb_tile[:],
                    bass.AP(
                        tensor=d_xb.tensor, offset=d_xb.offset,
                        ap=[[0, 128]] + d_xb.ap[1:],
                    ),
                )
            if d_cb is not None:
                cb_tile = consts.tile([128, DIM], F32, tag="cb")
                nc.gpsimd.dma_start(
                    cb_tile[:],
                    bass.AP(
                        tensor=d_cb.tensor, offset=d_cb.offset,
                        ap=[[0, 128]] + d_cb.ap[1:],
                    ),
                )

            # ---- persistent SBUF tensors ----
            xnT = persist.tile([128, KT, R], BF16, tag="xnT")
            kT = persist.tile([128, J], BF16, tag="kT")
            vo8 = [
                persist.tile([128, 2, VOW], FP8, tag=f"vo{t}", name=f"vo{t}")
                for t in range(CT // 2)
            ]
            qT = [
                persist.tile([128, R], BF16, tag=f"qT{h}", name=f"qT{h}")
                for h in range(HEADS // 2)
            ]
            aoT = [
                persist.tile([128, R], BF16, tag=f"aoT{k}", name=f"aoT{k}")
                for k in range(KT)
            ]
            aT = [
                persist.tile([128, R], BF16, tag=f"aT{f}", name=f"aT{f}")
                for f in range(FT)
            ]
            gT = [
                persist.tile([128, R], BF16, tag=f"gT{f}", name=f"gT{f}")
                for f in range(FT)
            ]
            wkv_sb = persist.tile([128, KT, 2 * DH], BF16, tag="wkv")
            wo_sb = wo_pool.tile([128, KT, DIM], BF16, tag="wosb")

            # ones column for the AV denominator
            for t in range(CT // 2):
                nc.vector.memset(vo8[t][:, :, DH : DH + 1], 1.0)

            with tc.tile_pool(name="wff1", bufs=2) as wff1_pool:
                # ---- FF1 unit generator (paced into phases C and D) ----
                w1_tiles = {}

                def w1_dma(g):
                    t = wff1_pool.tile([128, 2 * KT, 512], BF16, tag="w1g")
                    # halves in separate kk ranges: kk = half*8 + k
                    for half in range(2):
                        nc.sync.dma_start(
                            t[:, half * KT : (half + 1) * KT, :],
                            bass.AP(
                                tensor=d_wff1.tensor,
                                offset=d_wff1.offset + half * FF + g * 512,
                                ap=[[2 * FF, 128], [128 * 2 * FF, KT], [1, 512]],
                            ),
                        )
                    w1_tiles[g] = t

                def ff1_steps(fi):
                    g = fi // 4
                    fl = fi % 4
                    if fl == 0 and g + 2 < FT // 4:
                        w1_dma(g + 2)
                    w1 = w1_tiles[g]
                    a_ps = psum_f.tile([128, R], F32, tag="ffa")
                    g_ps = psum_f.tile([128, R], F32, tag="ffg")

                    def mk_chain(ps, kk0, k0):
                        def emit():
                            for k in range(k0, k0 + 4):
                                nc.tensor.matmul(
                                    ps[:],
                                    w1[:, kk0 + k, fl * 128 : (fl + 1) * 128],
                                    xnT[:, k, :],
                                    start=(k == 0),
                                    stop=(k == KT - 1),
                                )
                        return emit

                    def finish():
                        for k in range(4, 8):
                            nc.tensor.matmul(
                                g_ps[:],
                                w1[:, KT + k, fl * 128 : (fl + 1) * 128],
                                xnT[:, k, :],
                                start=False,
                                stop=(k == KT - 1),
                            )
                        nc.vector.tensor_copy(aT[fi][:], a_ps[:])
                        nc.vector.tensor_copy(gT[fi][:], g_ps[:])

                    return [
                        mk_chain(a_ps, 0, 0),
                        mk_chain(a_ps, 0, 4),
                        mk_chain(g_ps, KT, 0),
                        finish,
                    ]

                ff_queue = []
                ff_next = [0]

                def ff_step(n=1):
                    for _ in range(n):
                        if not ff_queue and ff_next[0] < FT:
                            ff_queue.extend(ff1_steps(ff_next[0]))
                            ff_next[0] += 1
                        if ff_queue:
                            ff_queue.pop(0)()

                with (
                    tc.tile_pool(name="ln", bufs=2) as ln_pool,
                    tc.tile_pool(name="stats", bufs=3) as stats_pool,
                    tc.tile_pool(name="psF", bufs=1, space="PSUM") as psum_f,
                ):
                    # ---- prologue DMAs (issue order matters per queue) ----
                    # gpsimd: x then ctx chunk 0
                    # sync: wq8, wkv, wff1 g0/g1, wo
                    def ln_stats(src2d, bias_tile):
                        """LN stats for one [128, DIM] f32 tile; returns
                        (rstd[128,1], mbias[128,1]) for ACT normalize."""
                        stats = stats_pool.tile(
                            [128, 2, nc.vector.BN_STATS_DIM], F32, tag="st"
                        )
                        nc.vector.bn_stats(stats[:, 0, :], src2d[:, 0:512])
                        nc.vector.bn_stats(stats[:, 1, :], src2d[:, 512:1024])
                        mv = stats_pool.tile([128, nc.vector.BN_AGGR_DIM], F32, tag="mv")
                        nc.vector.bn_aggr(mv[:], stats[:])
                        rstd = stats_pool.tile([128, 1], F32, tag="rs")
                        nc.scalar.activation(
                            rstd[:], mv[:, 1:2],
                            mybir.ActivationFunctionType.Sqrt, bias=eps_tile[:],
                        )
                        nc.vector.reciprocal(rstd[:], rstd[:])
                        mb = stats_pool.tile([128, 1], F32, tag="mb")
                        nc.vector.tensor_scalar(
                            out=mb[:], in0=mv[:, 0:1],
                            scalar1=rstd[:], scalar2=-1.0,
                            op0=mybir.AluOpType.mult, op1=mybir.AluOpType.mult,
                        )
                        return rstd, mb

                    def ln_normalize(dst, src2d, rstd, mb, bias_tile):
                        # dst = src*rstd - mu*rstd  (ACT Identity, per-row)
                        nc.scalar.activation(
                            dst, src2d,
                            mybir.ActivationFunctionType.Identity,
                            bias=mb[:], scale=rstd[:],
                        )
                        if bias_tile is not None:
                            nc.vector.tensor_add(dst, dst, bias_tile[:])

                    with (
                        tc.tile_pool(name="xload", bufs=1) as xload,
                        tc.tile_pool(name="wq8", bufs=1) as wq8_pool,
                        tc.tile_pool(name="psTR", bufs=2, space="PSUM") as psum_tr,
                        tc.tile_pool(name="psQ", bufs=2, space="PSUM") as psum_q,
                    ):
                        x_all = xload.tile([128, RT, DIM], F32, tag="xa")
                        nc.gpsimd.dma_start(x_all[:], _ap3(d_x, 128, RT, DIM))
                        wq8_sb = [
                            wq8_pool.tile(
                                [128, 2, INNER], FP8, tag=f"wq{p}", name=f"wq{p}"
                            )
                            for p in range(KP)
                        ]
                        for p in range(KP):
                            nc.sync.dma_start(
                                wq8_sb[p][:],
                                _ap3(d_wq8, 128, 2, INNER, off=2 * p * 128 * INNER),
                            )
                        nc.sync.dma_start(wkv_sb[:], _ap3(d_wkv, 128, KT, 2 * DH))
                        w1_dma(0)
                        w1_dma(1)
                        nc.sync.dma_start(wo_sb[:], _ap3(d_wo, 128, KT, DIM))
                        xnT8 = [
                            wq8_pool.tile(
                                [128, 2, R], FP8, tag=f"x8{p}", name=f"x8{p}"
                            )
                            for p in range(KP)
                        ]

                        # ---- Phase A: x layernorm + PE transposes ----
                        for t in range(RT):
                            rstd, mb = ln_stats(x_all[:, t, :], xb_tile)
                            xn_t = ln_pool.tile([128, DIM], BF16, tag="ln_xn")
                            ln_normalize(xn_t[:], x_all[:, t, :], rstd, mb, xb_tile)
                            for half in range(2):
                                tr = psum_tr.tile([128, 4, 128], BF16, tag="tr")
                                for q in range(4):
                                    k = half * 4 + q
                                    nc.tensor.transpose(
                                        tr[:, q, :],
                                        xn_t[:, k * 128 : (k + 1) * 128],
                                        ident[:],
                                    )
                                nc.vector.tensor_copy(
                                    xnT[:, half * 4 : (half + 1) * 4,
                                        t * 128 : (t + 1) * 128],
                                    tr[:],
                                )
                                # fp8 copies for the Q projection
                                nc.scalar.activation(
                                    xnT8[2 * half][:, :, t * 128 : (t + 1) * 128],
                                    tr[:, 0:2, :],
                                    mybir.ActivationFunctionType.Copy,
                                )
                                nc.scalar.activation(
                                    xnT8[2 * half + 1][:, :, t * 128 : (t + 1) * 128],
                                    tr[:, 2:4, :],
                                    mybir.ActivationFunctionType.Copy,
                                )

                        # ---- Phase B: Q projection (fp8 DoubleRow) ----
                        for hp in range(HEADS // 2):
                            q_ps = psum_q.tile([128, R], F32, tag="q")
                            for p in range(KP):
                                nc.tensor.matmul(
                                    q_ps[:],
                                    wq8_sb[p][:, :, hp * 128 : (hp + 1) * 128],
                                    xnT8[p][:],
                                    start=(p == 0),
                                    stop=(p == KP - 1),
                                    perf_mode=mybir.MatmulPerfMode.DoubleRow,
                                )
                            nc.vector.tensor_scalar(
                                out=qT[hp][:],
                                in0=q_ps[:],
                                scalar1=1.0 / QSC,
                                scalar2=None,
                                op0=mybir.AluOpType.mult,
                            )

                    # ---- Phase C: ctx layernorm + KV projection ----
                    with (
                        tc.tile_pool(name="cnload", bufs=2) as cnload,
                        tc.tile_pool(name="cnT", bufs=2) as cnT_pool,
                        tc.tile_pool(name="vstage", bufs=2) as vstage,
                        tc.tile_pool(name="psKV", bufs=2, space="PSUM") as psum_kv,
                        tc.tile_pool(name="psVT", bufs=2, space="PSUM") as psum_vt,
                    ):
                        cn_tiles = {}

                        def cn_dma(c):
                            t = cnload.tile([128, 4, DIM], F32, tag="cn")
                            nc.gpsimd.dma_start(
                                t[:], _ap3(d_ctx, 128, 4, DIM, off=c * 512 * DIM)
                            )
                            cn_tiles[c] = t

                        cn_dma(0)
                        cn_dma(1)
                        for c in range(J // 512):
                            if c + 2 < J // 512:
                                cn_dma(c + 2)
                            cnT = cnT_pool.tile([128, KT, 512], BF16, tag="cnT")
                            for t4 in range(4):
                                src = cn_tiles[c][:, t4, :]
                                rstd, mb = ln_stats(src, cb_tile)
                                cn_t = ln_pool.tile([128, DIM], BF16, tag="ln_cn")
                                ln_normalize(cn_t[:], src, rstd, mb, cb_tile)
                                nc.sync.dma_start_transpose(
                                    cnT[:, :, t4 * 128 : (t4 + 1) * 128], cn_t[:]
                                )
                                ff_step(1)
                            kv_ps = psum_kv.tile([128, 512], F32, tag="kv")
                            for k in range(KT):
                                nc.tensor.matmul(
                                    kv_ps[:],
                                    wkv_sb[:, k, :],
                                    cnT[:, k, :],
                                    start=(k == 0),
                                    stop=(k == KT - 1),
                                )
                            nc.vector.tensor_copy(
                                kT[0:DH, c * 512 : (c + 1) * 512], kv_ps[0:DH, :]
                            )
                            nc.gpsimd.dma_start(
                                kT[DH:128, c * 512 : (c + 1) * 512],
                                kT[0:DH, c * 512 : (c + 1) * 512],
                            )
                            vT_sb = vstage.tile([128, 512], BF16, tag="vT")
                            nc.vector.tensor_copy(vT_sb[DH:128, :], kv_ps[DH:128, :])
                            for j4 in range(4):
                                jc = c * 4 + j4
                                vps = psum_vt.tile([128, DH], BF16, tag="vtr")
                                nc.tensor.transpose(
                                    vps[:],
                                    vT_sb[DH:128, j4 * 128 : (j4 + 1) * 128],
                                    ident[DH:128, DH:128],
                                )
                                nc.vector.tensor_copy(
                                    vo8[jc // 2][:, jc % 2, 0:DH], vps[:]
                                )
                            ff_step(2)

                    # ---- Phase D: attention (sim row-packed, AV fp8 DR) ----
                    with (
                        tc.tile_pool(name="attn", bufs=16) as a_pool,
                        tc.tile_pool(name="smx", bufs=3) as smx_pool,
                        tc.tile_pool(name="psS", bufs=2, space="PSUM") as psum_s,
                        tc.tile_pool(name="psAV", bufs=2, space="PSUM") as psum_av,
                    ):
                        for hp in range(HEADS // 2):
                            a_sbs = [[], []]
                            for p in range(CT // 2):
                                s0 = psum_s.tile([128, 2, R], F32, tag="s")
                                s1 = psum_s.tile([128, 2, R], F32, tag="s")
                                for half in range(2):
                                    jc = 2 * p + half
                                    nc.tensor.matmul(
                                        s0[:, half, :],
                                        kT[0:DH, jc * 128 : (jc + 1) * 128],
                                        qT[hp][0:DH, :],
                                        start=True,
                                        stop=True,
                                    )
                                    nc.tensor.matmul(
                                        s1[:, half, :],
                                        kT[DH:128, jc * 128 : (jc + 1) * 128],
                                        qT[hp][DH:128, :],
                                        start=True,
                                        stop=True,
                                    )
                                for h2, s in ((0, s0), (1, s1)):
                                    a_t = a_pool.tile([128, 2, R], FP8, tag="a")
                                    nc.scalar.activation(
                                        a_t[:], s[:],
                                        mybir.ActivationFunctionType.Exp,
                                    )
                                    a_sbs[h2].append(a_t)
                                ff_step(2)
                            av = [
                                psum_av.tile([DH + 1, R], F32, tag="av", name=f"av{h2}")
                                for h2 in range(2)
                            ]
                            for p in range(CT // 2):
                                for h2 in range(2):
                                    nc.tensor.matmul(
                                        av[h2][:],
                                        vo8[p][:, :, 0 : DH + 1],
                                        a_sbs[h2][p][:],
                                        start=(p == 0),
                                        stop=(p == CT // 2 - 1),
                                        perf_mode=mybir.MatmulPerfMode.DoubleRow,
                                    )
                                if p in (2, 5):
                                    ff_step(1)
                            for h2 in range(2):
                                rec = smx_pool.tile([DH + 1, R], F32, tag="rec")
                                nc.vector.reciprocal(
                                    rec[DH : DH + 1, :], av[h2][DH : DH + 1, :]
                                )
                                r0 = smx_pool.tile([1, R], F32, tag="r0")
                                nc.gpsimd.dma_start(r0[:], rec[DH : DH + 1, :])
                                rbc = smx_pool.tile([DH, R], F32, tag="rbc")
                                nc.gpsimd.partition_broadcast(rbc[:], r0[:])
                                if h2 == 0:
                                    nc.vector.tensor_mul(
                                        aoT[hp][0:DH, :], av[h2][0:DH, :], rbc[:]
                                    )
                                else:
                                    tmp = smx_pool.tile([DH, R], BF16, tag="tmp")
                                    nc.vector.tensor_mul(
                                        tmp[:], av[h2][0:DH, :], rbc[:]
                                    )
                                    nc.gpsimd.dma_start(aoT[hp][DH:128, :], tmp[:])
                            ff_step(2)
                        while ff_queue or ff_next[0] < FT:
                            ff_step(1)

            # ---- Phase E: Wo + SwiGLU + FF2, N=1024 rhs, 4 row-tile psums ----
            with (
                tc.tile_pool(name="wff2", bufs=2) as wff2_pool,
                tc.tile_pool(name="hstage", bufs=3) as hstage,
                tc.tile_pool(name="psO", bufs=1, space="PSUM") as psum_o,
            ):
                w2_tiles = {}

                def w2_dma(g):
                    t = wff2_pool.tile([128, 8, DIM], BF16, tag="w2")
                    nc.sync.dma_start(
                        t[:], _ap3(d_wff2, 128, 8, DIM, off=g * 8 * 128 * DIM)
                    )
                    w2_tiles[g] = t

                h_next = [0]

                def h_step(n=1):
                    for _ in range(n):
                        fi = h_next[0]
                        if fi >= FT:
                            return
                        h_next[0] += 1
                        sil = hstage.tile([128, R], F32, tag="sil")
                        nc.scalar.activation(
                            sil[:], gT[fi][:],
                            mybir.ActivationFunctionType.Silu,
                        )
                        nc.vector.tensor_mul(gT[fi][:], aT[fi][:], sil[:])

                w2_dma(0)
                for ch in range(2):
                    o_ps = [
                        psum_o.tile(
                            [128, 512], F32, tag=f"o{ch}{rs}", name=f"o{ch}{rs}"
                        )
                        for rs in range(RT)
                    ]
                    for k in range(KT):
                        for rs in range(RT):
                            nc.tensor.matmul(
                                o_ps[rs][:],
                                aoT[k][:, rs * 128 : (rs + 1) * 128],
                                wo_sb[:, k, ch * 512 : (ch + 1) * 512],
                                start=(k == 0),
                                stop=False,
                            )
                        if ch == 0 and k == 1:
                            w2_dma(1)
                        if ch == 0 and k >= 4:
                            h_step(1)
                    for g in range(4):
                        for f8 in range(8):
                            fi = g * 8 + f8
                            if ch == 0:
                                h_step(1)
                            for rs in range(RT):
                                nc.tensor.matmul(
                                    o_ps[rs][:],
                                    gT[fi][:, rs * 128 : (rs + 1) * 128],
                                    w2_tiles[g][:, f8, ch * 512 : (ch + 1) * 512],
                                    start=False,
                                    stop=(fi == FT - 1),
                                )
                        if ch == 0 and g + 2 < 4:
                            w2_dma(g + 2)
                    for rs in range(RT):
                        o_sb = hstage.tile([128, 512], F32, tag="ost")
                        nc.vector.tensor_copy(o_sb[:], o_ps[rs][:])
                        nc.gpsimd.dma_start(
                            d_out[
                                rs * 128 : (rs + 1) * 128,
                                ch * 512 : (ch + 1) * 512,
                            ],
                            o_sb[:],
                        )

    nc.compile()
    return nc


_NC_CACHE = {}


def _get_nc(x_bias_nonzero, c_bias_nonzero):
    key = (x_bias_nonzero, c_bias_nonzero)
    if key not in _NC_CACHE:
        _NC_CACHE[key] = build_kernel(*key)
    return _NC_CACHE[key]


def make_in_maps(x, context, norm_g, norm_b, cnorm_g, cnorm_b, Wq, Wkv, Wo, Wff1, Wff2):
    x = np.asarray(x, np.float32)
    context = np.asarray(context, np.float32)
    norm_g = np.asarray(norm_g, np.float32)
    norm_b = np.asarray(norm_b, np.float32)
    cnorm_g = np.asarray(cnorm_g, np.float32)
    cnorm_b = np.asarray(cnorm_b, np.float32)
    scale = DH ** -0.5
    bf = ml_dtypes.bfloat16
    f8 = ml_dtypes.float8_e4m3
    wq8 = np.ascontiguousarray(
        np.clip(
            norm_g[:, None] * np.asarray(Wq, np.float32) * scale * QSC, -240, 240
        )
    ).astype(f8)
    wkv = np.ascontiguousarray(cnorm_g[:, None] * np.asarray(Wkv, np.float32)).astype(bf)
    wo = np.ascontiguousarray(np.asarray(Wo, np.float32)).astype(bf)
    wff1 = np.ascontiguousarray(norm_g[:, None] * np.asarray(Wff1, np.float32)).astype(bf)
    wff2 = np.ascontiguousarray(np.asarray(Wff2, np.float32)).astype(bf)
    x_bias = bool(np.any(norm_b != 0.0))
    c_bias = bool(np.any(cnorm_b != 0.0))
    in_maps = []
    for c in range(N_CORES):
        b = c // (N_CORES // B)
        r0 = (c % (N_CORES // B)) * R
        m = {
            "x": np.ascontiguousarray(x[b, r0 : r0 + R, :]),
            "ctx": np.ascontiguousarray(context[b]),
            "wq8": wq8,
            "wkv": wkv,
            "wo": wo,
            "wff1": wff1,
            "wff2": wff2,
        }
        if x_bias:
            m["xb"] = norm_b.reshape(1, DIM).copy()
        if c_bias:
            m["cb"] = cnorm_b.reshape(1, DIM).copy()
        in_maps.append(m)
    return in_maps, x_bias, c_bias


def gather_output(results):
    out = np.empty((B, N, DIM), np.float32)
    for c in range(N_CORES):
        b = c // (N_CORES // B)
        r0 = (c % (N_CORES // B)) * R
        out[b, r0 : r0 + R, :] = results[c]["out"]
    return out


def kernel(**inputs):
    from concourse.bass_utils import run_bass_kernel_spmd

    in_maps, x_bias, c_bias = make_in_maps(**inputs)
    nc = _get_nc(x_bias, c_bias)
    res = run_bass_kernel_spmd(nc, in_maps, list(range(N_CORES)))
    return gather_output(res.results)


# revision 19
# speedup vs baseline: 1.0264x; 1.0094x over previous
"""Trainium2 Bass kernel for nn_CrossAttention_65051574665735.

Cross-attention block (MQA, shared K/V head) + parallel SwiGLU FF.
Data-parallel over B*N rows across 8 NeuronCores: core c handles batch c//4,
rows (c%4)*512. Context + weights replicated (weights pre-cast to bf16/fp8
with the layernorm scale g and the 1/sqrt(dh) attention scale folded in on
the host). No cross-core collectives; the host concatenates the 8 slices.

Schedule notes (v2):
- Weight/x/ctx DMAs are batched into a handful of large multi-dim
  descriptors (sync queue for weights + ctx transposes, gpsimd for x/ctx
  loads and stores) so queue-issue time stops being a bottleneck.
- x layernorm transposes run on the (otherwise idle) PE via identity
  matmuls; LN normalize runs on ACT (Identity with per-row scale+bias)
  so DVE only does stats + copies.
- sim matmuls for the two heads of a pair are issued back-to-back into
  disjoint PE row groups (K=64 each) so they execute concurrently.
- softmax probs are written as fp8e4 directly by the ACT exp; AV runs as
  fp8 DoubleRow over j-tile pairs (vo pair tiles [128,2,80], ones column
  at col 64 gives the softmax denominator for free).
- FF1 up-proj matmuls are emitted in small "steps" interleaved into the
  ctx and attention phases to fill PE gaps; SwiGLU (silu*a) is deferred
  into phase E where ACT is free, h overwrites gT in place.
- Phase E streams Wo + FF2 with N=1024 rhs into 4 row-tile PSUM banks and
  stores results straight from PSUM to DRAM.
"""

import sys

if "/opt/trn_rl_repo" not in sys.path:
    sys.path.insert(0, "/opt/trn_rl_repo")

import numpy as np
import ml_dtypes

import concourse.bass as bass
import concourse.tile as tile
from concourse import mybir, bacc
from concourse.masks import make_identity

F32 = mybir.dt.float32
BF16 = mybir.dt.bfloat16
FP8 = mybir.dt.float8e4

B, N, J = 2, 2048, 2048
DIM, HEADS, DH = 1024, 16, 64
INNER = HEADS * DH
FF = 4 * DIM
EPS = 1e-5
N_CORES = 8
R = B * N // N_CORES  # 512 rows per core
KT = DIM // 128  # 8 contraction tiles over dim
KP = KT // 2  # 4 fp8 contraction pairs
RT = R // 128  # 4 row tiles
CT = J // 128  # 16 context row tiles
FT = FF // 128  # 32 ff tiles
QSC = 256.0  # host pre-scale on Wq for fp8
VOW = 80  # vo pair tile free width (65 used; 80 for DR step%16==0)


def _ap3(d, n0, n1, n2, off=0):
    """3D view [n0 part, n1, n2] over a 2D dram tensor d with row length n2."""
    return bass.AP(
        tensor=d.tensor,
        offset=d.offset + off,
        ap=[[n2, n0], [n0 * n2, n1], [1, n2]],
    )


def build_kernel(x_bias_nonzero: bool, c_bias_nonzero: bool):
    nc = bacc.Bacc(
        "TRN2", target_bir_lowering=False, debug=False, num_devices=N_CORES
    )
    d_x = nc.dram_tensor("x", [R, DIM], F32, kind="ExternalInput").ap()
    d_ctx = nc.dram_tensor("ctx", [J, DIM], F32, kind="ExternalInput").ap()
    d_wq = nc.dram_tensor("wq", [DIM, INNER], BF16, kind="ExternalInput").ap()
    d_wkv = nc.dram_tensor("wkv", [DIM, 2 * DH], BF16, kind="ExternalInput").ap()
    d_wo = nc.dram_tensor("wo", [INNER, DIM], BF16, kind="ExternalInput").ap()
    d_wff1 = nc.dram_tensor("wff1", [DIM, 2 * FF], BF16, kind="ExternalInput").ap()
    d_wff2 = nc.dram_tensor("wff2", [FF, DIM], BF16, kind="ExternalInput").ap()
    d_xb = (
        nc.dram_tensor("xb", [1, DIM], F32, kind="ExternalInput").ap()
        if x_bias_nonzero
        else None
    )
    d_cb = (
        nc.dram_tensor("cb", [1, DIM], F32, kind="ExternalInput").ap()
        if c_bias_nonzero
        else None
    )
    d_out = nc.dram_tensor("out", [R, DIM], F32, kind="ExternalOutput").ap()

    with tile.TileContext(nc) as tc:
        with (
            tc.tile_pool(name="consts", bufs=1) as consts,
            tc.tile_pool(name="persist", bufs=1) as persist,
            tc.tile_pool(name="wo", bufs=1) as wo_pool,
        ):
            ident = consts.tile([128, 128], BF16)
            make_identity(nc, ident)
            eps_tile = consts.tile([128, 1], F32, tag="eps")
            nc.vector.memset(eps_tile[:], EPS)

            xb_tile = cb_tile = None
            if d_xb is not None:
                xb_tile = consts.tile([128, DIM], F32, tag="xb")
                nc.gpsimd.dma_start(
                    xb_tile[:],
                    bass.AP(
                        tensor=d_xb.tensor, offset=d_xb.offset,
                        ap=[[0, 128]] + d_xb.ap[1:],
                    ),
                )
            if d_cb is not None:
                cb_tile = consts.tile([128, DIM], F32, tag="cb")
                nc.gpsimd.dma_start(
                    cb_tile[:],
                    bass.AP(
                        tensor=d_cb.tensor, offset=d_cb.offset,
                        ap=[[0, 128]] + d_cb.ap[1:],
                    ),
                )

            # ---- persistent SBUF tensors ----
            xnT = persist.tile([128, KT, R], BF16, tag="xnT")
            kT = persist.tile([128, J], BF16, tag="kT")
            vo8 = [
                persist.tile([128, 2, VOW], FP8, tag=f"vo{t}", name=f"vo{t}")
                for t in range(CT // 2)
            ]
            qT = [
                persist.tile([128, R], BF16, tag=f"qT{h}", name=f"qT{h}")
                for h in range(HEADS // 2)
            ]
            aoT = [
                persist.tile([128, R], BF16, tag=f"aoT{k}", name=f"aoT{k}")
                for k in range(KT)
            ]
            aT = [
                persist.tile([128, R], BF16, tag=f"aT{f}", name=f"aT{f}")
                for f in range(FT)
            ]
            gT = [
                persist.tile([128, R], BF16, tag=f"gT{f}", name=f"gT{f}")
                for f in range(FT)
            ]
            wkv_sb = persist.tile([128, KT, 2 * DH], BF16, tag="wkv")
            wo_sb = wo_pool.tile([128, KT, DIM], BF16, tag="wosb")

            # ones column for the AV denominator
            for t in range(CT // 2):
                nc.vector.memset(vo8[t][:, :, DH : DH + 1], 1.0)

            with tc.tile_pool(name="wff1", bufs=2) as wff1_pool:
                # ---- FF1 unit generator (paced into phases C and D) ----
                w1_tiles = {}

                def w1_dma(g):
                    t = wff1_pool.tile([128, 2 * KT, 512], BF16, tag="w1g")
                    # halves in separate kk ranges: kk = half*8 + k
                    for half in range(2):
                        nc.gpsimd.dma_start(
                            t[:, half * KT : (half + 1) * KT, :],
                            bass.AP(
                                tensor=d_wff1.tensor,
                                offset=d_wff1.offset + half * FF + g * 512,
                                ap=[[2 * FF, 128], [128 * 2 * FF, KT], [1, 512]],
                            ),
                        )
                    w1_tiles[g] = t

                def ff1_steps(fi):
                    g = fi // 4
                    fl = fi % 4
                    if fl == 0 and g + 2 < FT // 4:
                        w1_dma(g + 2)
                    w1 = w1_tiles[g]
                    a_ps = psum_f.tile([128, R], F32, tag="ffa")
                    g_ps = psum_f.tile([128, R], F32, tag="ffg")

                    def mk_chain(ps, kk0, k0):
                        def emit():
                            for k in range(k0, k0 + 4):
                                nc.tensor.matmul(
                                    ps[:],
                                    w1[:, kk0 + k, fl * 128 : (fl + 1) * 128],
                                    xnT[:, k, :],
                                    start=(k == 0),
                                    stop=(k == KT - 1),
                                )
                        return emit

                    def finish():
                        for k in range(4, 8):
                            nc.tensor.matmul(
                                g_ps[:],
                                w1[:, KT + k, fl * 128 : (fl + 1) * 128],
                                xnT[:, k, :],
                                start=False,
                                stop=(k == KT - 1),
                            )
                        nc.vector.tensor_copy(aT[fi][:], a_ps[:])
                        nc.vector.tensor_copy(gT[fi][:], g_ps[:])

                    return [
                        mk_chain(a_ps, 0, 0),
                        mk_chain(a_ps, 0, 4),
                        mk_chain(g_ps, KT, 0),
                        finish,
                    ]

                ff_queue = []
                ff_next = [0]

                def ff_step(n=1):
                    for _ in range(n):
                        if not ff_queue and ff_next[0] < FT:
                            ff_queue.extend(ff1_steps(ff_next[0]))
                            ff_next[0] += 1
                        if ff_queue:
                            ff_queue.pop(0)()

                with (
                    tc.tile_pool(name="ln", bufs=2) as ln_pool,
                    tc.tile_pool(name="stats", bufs=3) as stats_pool,
                    tc.tile_pool(name="psF", bufs=1, space="PSUM") as psum_f,
                ):
                    # ---- prologue DMAs (issue order matters per queue) ----
                    # gpsimd: x then ctx chunk 0
                    # sync: wq8, wkv, wff1 g0/g1, wo
                    def ln_stats(src2d, bias_tile):
                        """LN stats for one [128, DIM] f32 tile; returns
                        (rstd[128,1], mbias[128,1]) for ACT normalize."""
                        stats = stats_pool.tile(
                            [128, 2, nc.vector.BN_STATS_DIM], F32, tag="st"
                        )
                        nc.vector.bn_stats(stats[:, 0, :], src2d[:, 0:512])
                        nc.vector.bn_stats(stats[:, 1, :], src2d[:, 512:1024])
                        mv = stats_pool.tile([128, nc.vector.BN_AGGR_DIM], F32, tag="mv")
                        nc.vector.bn_aggr(mv[:], stats[:])
                        rstd = stats_pool.tile([128, 1], F32, tag="rs")
                        nc.scalar.activation(
                            rstd[:], mv[:, 1:2],
                            mybir.ActivationFunctionType.Sqrt, bias=eps_tile[:],
                        )
                        nc.vector.reciprocal(rstd[:], rstd[:])
                        mb = stats_pool.tile([128, 1], F32, tag="mb")
                        nc.vector.tensor_scalar(
                            out=mb[:], in0=mv[:, 0:1],
                            scalar1=rstd[:], scalar2=-1.0,
                            op0=mybir.AluOpType.mult, op1=mybir.AluOpType.mult,
                        )
                        return rstd, mb

                    def ln_normalize(dst, src2d, rstd, mb, bias_tile):
                        # dst = src*rstd - mu*rstd  (ACT Identity, per-row)
                        nc.scalar.activation(
                            dst, src2d,
                            mybir.ActivationFunctionType.Identity,
                            bias=mb[:], scale=rstd[:],
                        )
                        if bias_tile is not None:
                            nc.vector.tensor_add(dst, dst, bias_tile[:])

                    cnload = tc.alloc_tile_pool(name="cnload", bufs=5)
                    cn_tiles = {}

                    def cn_dma(j):
                        t = cnload.tile([128, DIM], F32, tag="cn", name=f"cn{j}")
                        nc.gpsimd.dma_start(t[:], d_ctx[j * 128 : (j + 1) * 128, :])
                        cn_tiles[j] = t

                    with (
                        tc.tile_pool(name="xload", bufs=4) as xload,
                        tc.tile_pool(name="wq", bufs=1) as wq_pool,
                        tc.tile_pool(name="psTR", bufs=2, space="PSUM") as psum_tr,
                        tc.tile_pool(name="psQ", bufs=2, space="PSUM") as psum_q,
                    ):
                        x_ts = []
                        for t in range(RT):
                            x_t = xload.tile([128, DIM], F32, tag="xa", name=f"x{t}")
                            nc.gpsimd.dma_start(
                                x_t[:], d_x[t * 128 : (t + 1) * 128, :]
                            )
                            x_ts.append(x_t)
                        for j in range(4):
                            cn_dma(j)
                        wq_sb = wq_pool.tile([128, KT, INNER], BF16, tag="wq")
                        nc.sync.dma_start(wq_sb[:], _ap3(d_wq, 128, KT, INNER))

                        # ---- Phase A: x layernorm + PE transposes ----
                        for t in range(RT):
                            rstd, mb = ln_stats(x_ts[t][:, :], xb_tile)
                            xn_t = ln_pool.tile([128, DIM], BF16, tag="ln_xn")
                            ln_normalize(xn_t[:], x_ts[t][:, :], rstd, mb, xb_tile)
                            for half in range(2):
                                tr = psum_tr.tile([128, 4, 128], BF16, tag="tr")
                                for q in range(4):
                                    k = half * 4 + q
                                    nc.tensor.transpose(
                                        tr[:, q, :],
                                        xn_t[:, k * 128 : (k + 1) * 128],
                                        ident[:],
                                    )
                                nc.vector.tensor_copy(
                                    xnT[:, half * 4 : (half + 1) * 4,
                                        t * 128 : (t + 1) * 128],
                                    tr[:],
                                )

                        # weight prefetches behind x/ctx loads on gpsimd
                        w1_dma(0)
                        w1_dma(1)
                        nc.gpsimd.dma_start(
                            wkv_sb[:], _ap3(d_wkv, 128, KT, 2 * DH)
                        )
                        nc.gpsimd.dma_start(wo_sb[:], _ap3(d_wo, 128, KT, DIM))

                        # ---- Phase B: Q projection (bf16) ----
                        for hp in range(HEADS // 2):
                            q_ps = psum_q.tile([128, R], F32, tag="q")
                            for k in range(KT):
                                nc.tensor.matmul(
                                    q_ps[:],
                                    wq_sb[:, k, hp * 128 : (hp + 1) * 128],
                                    xnT[:, k, :],
                                    start=(k == 0),
                                    stop=(k == KT - 1),
                                )
                            nc.vector.tensor_copy(qT[hp][:], q_ps[:])

                    # ---- Phase C: ctx layernorm + KV projection ----
                    with (
                        tc.tile_pool(name="cnT", bufs=2) as cnT_pool,
                        tc.tile_pool(name="vstage", bufs=2) as vstage,
                        tc.tile_pool(name="psKV", bufs=2, space="PSUM") as psum_kv,
                        tc.tile_pool(name="psVT", bufs=2, space="PSUM") as psum_vt,
                    ):
                        for c in range(J // 512):
                            cnT = cnT_pool.tile([128, KT, 512], BF16, tag="cnT")
                            for t4 in range(4):
                                j = 4 * c + t4
                                if j + 4 < CT:
                                    cn_dma(j + 4)
                                src = cn_tiles[j][:, :]
                                rstd, mb = ln_stats(src, cb_tile)
                                cn_t = ln_pool.tile([128, DIM], BF16, tag="ln_cn")
                                ln_normalize(cn_t[:], src, rstd, mb, cb_tile)
                                nc.sync.dma_start_transpose(
                                    cnT[:, :, t4 * 128 : (t4 + 1) * 128], cn_t[:]
                                )
                                ff_step(2)
                            kv_ps = psum_kv.tile([128, 512], F32, tag="kv")
                            for k in range(KT):
                                nc.tensor.matmul(
                                    kv_ps[:],
                                    wkv_sb[:, k, :],
                                    cnT[:, k, :],
                                    start=(k == 0),
                                    stop=(k == KT - 1),
                                )
                            nc.vector.tensor_copy(
                                kT[0:DH, c * 512 : (c + 1) * 512], kv_ps[0:DH, :]
                            )
                            nc.gpsimd.dma_start(
                                kT[DH:128, c * 512 : (c + 1) * 512],
                                kT[0:DH, c * 512 : (c + 1) * 512],
                            )
                            vT_sb = vstage.tile([128, 512], BF16, tag="vT")
                            nc.vector.tensor_copy(vT_sb[DH:128, :], kv_ps[DH:128, :])
                            for j4 in range(4):
                                jc = c * 4 + j4
                                vps = psum_vt.tile([128, DH], BF16, tag="vtr")
                                nc.tensor.transpose(
                                    vps[:],
                                    vT_sb[DH:128, j4 * 128 : (j4 + 1) * 128],
                                    ident[DH:128, DH:128],
                                )
                                nc.vector.tensor_copy(
                                    vo8[jc // 2][:, jc % 2, 0:DH], vps[:]
                                )
                            ff_step(2)

                    cnload.release()

                    # ---- Phase D: attention (sim row-packed, AV fp8 DR) ----
                    with (
                        tc.tile_pool(name="attn", bufs=16) as a_pool,
                        tc.tile_pool(name="smx", bufs=3) as smx_pool,
                        tc.tile_pool(name="psS", bufs=2, space="PSUM") as psum_s,
                        tc.tile_pool(name="psAV", bufs=2, space="PSUM") as psum_av,
                    ):
                        for hp in range(HEADS // 2):
                            a_sbs = [[], []]
                            for p in range(CT // 2):
                                s0 = psum_s.tile([128, 2, R], F32, tag="s")
                                s1 = psum_s.tile([128, 2, R], F32, tag="s")
                                for half in range(2):
                                    jc = 2 * p + half
                                    nc.tensor.matmul(
                                        s0[:, half, :],
                                        kT[0:DH, jc * 128 : (jc + 1) * 128],
                                        qT[hp][0:DH, :],
                                        start=True,
                                        stop=True,
                                    )
                                    nc.tensor.matmul(
                                        s1[:, half, :],
                                        kT[DH:128, jc * 128 : (jc + 1) * 128],
                                        qT[hp][DH:128, :],
                                        start=True,
                                        stop=True,
                                    )
                                for h2, s in ((0, s0), (1, s1)):
                                    a_t = a_pool.tile([128, 2, R], FP8, tag="a")
                                    nc.scalar.activation(
                                        a_t[:], s[:],
                                        mybir.ActivationFunctionType.Exp,
                                    )
                                    a_sbs[h2].append(a_t)
                                ff_step(2)
                            av = [
                                psum_av.tile([DH + 1, R], F32, tag="av", name=f"av{h2}")
                                for h2 in range(2)
                            ]
                            for p in range(CT // 2):
                                for h2 in range(2):
                                    nc.tensor.matmul(
                                        av[h2][:],
                                        vo8[p][:, :, 0 : DH + 1],
                                        a_sbs[h2][p][:],
                                        start=(p == 0),
                                        stop=(p == CT // 2 - 1),
                                        perf_mode=mybir.MatmulPerfMode.DoubleRow,
                                    )
                                if p in (2, 5):
                                    ff_step(1)
                            for h2 in range(2):
                                rec = smx_pool.tile([DH + 1, R], F32, tag="rec")
                                nc.vector.tensor_copy(
                                    rec[DH : DH + 1, :], av[h2][DH : DH + 1, :]
                                )
                                r0 = smx_pool.tile([1, R], F32, tag="r0")
                                nc.gpsimd.dma_start(r0[:], rec[DH : DH + 1, :])
                                rbc = smx_pool.tile([DH, R], F32, tag="rbc")
                                nc.gpsimd.partition_broadcast(rbc[:], r0[:])
                                nc.vector.reciprocal_approx_fast(rbc[:], rbc[:])
                                if h2 == 0:
                                    nc.vector.tensor_mul(
                                        aoT[hp][0:DH, :], av[h2][0:DH, :], rbc[:]
                                    )
                                else:
                                    tmp = smx_pool.tile([DH, R], BF16, tag="tmp")
                                    nc.vector.tensor_mul(
                                        tmp[:], av[h2][0:DH, :], rbc[:]
                                    )
                                    nc.gpsimd.dma_start(aoT[hp][DH:128, :], tmp[:])
                            ff_step(2)
                        while ff_queue or ff_next[0] < FT:
                            ff_step(1)

            # ---- Phase E: Wo + SwiGLU + FF2, N=1024 rhs, 4 row-tile psums ----
            with (
                tc.tile_pool(name="wff2", bufs=2) as wff2_pool,
                tc.tile_pool(name="hstage", bufs=3) as hstage,
                tc.tile_pool(name="psO", bufs=1, space="PSUM") as psum_o,
            ):
                w2_tiles = {}

                def w2_dma(g):
                    t = wff2_pool.tile([128, 8, DIM], BF16, tag="w2")
                    nc.sync.dma_start(
                        t[:], _ap3(d_wff2, 128, 8, DIM, off=g * 8 * 128 * DIM)
                    )
                    w2_tiles[g] = t

                h_next = [0]

                def h_step(n=1):
                    for _ in range(n):
                        fi = h_next[0]
                        if fi >= FT:
                            return
                        h_next[0] += 1
                        sil = hstage.tile([128, R], F32, tag="sil")
                        nc.scalar.activation(
                            sil[:], gT[fi][:],
                            mybir.ActivationFunctionType.Silu,
                        )
                        nc.vector.tensor_mul(gT[fi][:], aT[fi][:], sil[:])

                w2_dma(0)
                for ch in range(2):
                    o_ps = [
                        psum_o.tile(
                            [128, 512], F32, tag=f"o{ch}{rs}", name=f"o{ch}{rs}"
                        )
                        for rs in range(RT)
                    ]
                    for k in range(KT):
                        for rs in range(RT):
                            nc.tensor.matmul(
                                o_ps[rs][:],
                                aoT[k][:, rs * 128 : (rs + 1) * 128],
                                wo_sb[:, k, ch * 512 : (ch + 1) * 512],
                                start=(k == 0),
                                stop=False,
                            )
                        if ch == 0 and k == 1:
                            w2_dma(1)
                        if ch == 0 and k >= 4:
                            h_step(1)
                    for g in range(4):
                        for f8 in range(8):
                            fi = g * 8 + f8
                            if ch == 0:
                                h_step(1)
                            for rs in range(RT):
                                nc.tensor.matmul(
                                    o_ps[rs][:],
                                    gT[fi][:, rs * 128 : (rs + 1) * 128],
                                    w2_tiles[g][:, f8, ch * 512 : (ch + 1) * 512],
                                    start=False,
                                    stop=(fi == FT - 1),
                                )
                        if ch == 0 and g + 2 < 4:
                            w2_dma(g + 2)
                    for rs in range(RT):
                        o_sb = hstage.tile([128, 512], F32, tag="ost")
                        nc.scalar.activation(
                            o_sb[:], o_ps[rs][:],
                            mybir.ActivationFunctionType.Copy,
                        )
                        nc.gpsimd.dma_start(
                            d_out[
                                rs * 128 : (rs + 1) * 128,
                                ch * 512 : (ch + 1) * 512,
                            ],
                            o_sb[:],
                        )

    nc.compile()
    return nc


_NC_CACHE = {}


def _get_nc(x_bias_nonzero, c_bias_nonzero):
    key = (x_bias_nonzero, c_bias_nonzero)
    if key not in _NC_CACHE:
        _NC_CACHE[key] = build_kernel(*key)
    return _NC_CACHE[key]


def make_in_maps(x, context, norm_g, norm_b, cnorm_g, cnorm_b, Wq, Wkv, Wo, Wff1, Wff2):
    x = np.asarray(x, np.float32)
    context = np.asarray(context, np.float32)
    norm_g = np.asarray(norm_g, np.float32)
    norm_b = np.asarray(norm_b, np.float32)
    cnorm_g = np.asarray(cnorm_g, np.float32)
    cnorm_b = np.asarray(cnorm_b, np.float32)
    scale = DH ** -0.5
    bf = ml_dtypes.bfloat16
    f8 = ml_dtypes.float8_e4m3
    wq = np.ascontiguousarray(
        norm_g[:, None] * np.asarray(Wq, np.float32) * scale
    ).astype(bf)
    wkv = np.ascontiguousarray(cnorm_g[:, None] * np.asarray(Wkv, np.float32)).astype(bf)
    wo = np.ascontiguousarray(np.asarray(Wo, np.float32)).astype(bf)
    wff1 = np.ascontiguousarray(norm_g[:, None] * np.asarray(Wff1, np.float32)).astype(bf)
    wff2 = np.ascontiguousarray(np.asarray(Wff2, np.float32)).astype(bf)
    x_bias = bool(np.any(norm_b != 0.0))
    c_bias = bool(np.any(cnorm_b != 0.0))
    in_maps = []
    for c in range(N_CORES):
        b = c // (N_CORES // B)
        r0 = (c % (N_CORES // B)) * R
        m = {
            "x": np.ascontiguousarray(x[b, r0 : r0 + R, :]),
            "ctx": np.ascontiguousarray(context[b]),
            "wq": wq,
            "wkv": wkv,
            "wo": wo,
            "wff1": wff1,
            "wff2": wff2,
        }
        if x_bias:
            m["xb"] = norm_b.reshape(1, DIM).copy()
        if c_bias:
            m["cb"] = cnorm_b.reshape(1, DIM).copy()
        in_maps.append(m)
    return in_maps, x_bias, c_bias


def gather_output(results):
    out = np.empty((B, N, DIM), np.float32)
    for c in range(N_CORES):
        b = c // (N_CORES // B)
        r0 = (c % (N_CORES // B)) * R
        out[b, r0 : r0 + R, :] = results[c]["out"]
    return out


def kernel(**inputs):
    from concourse.bass_utils import run_bass_kernel_spmd

    in_maps, x_bias, c_bias = make_in_maps(**inputs)
    nc = _get_nc(x_bias, c_bias)
    res = run_bass_kernel_spmd(nc, in_maps, list(range(N_CORES)))
    return gather_output(res.results)


# revision 25
# speedup vs baseline: 1.0369x; 1.0103x over previous
"""Trainium2 Bass kernel for nn_CrossAttention_65051574665735.

Cross-attention block (MQA, shared K/V head) + parallel SwiGLU FF.
Data-parallel over B*N rows across 8 NeuronCores: core c handles batch c//4,
rows (c%4)*512. Context + weights replicated (weights pre-cast to bf16/fp8
with the layernorm scale g and the 1/sqrt(dh) attention scale folded in on
the host). No cross-core collectives; the host concatenates the 8 slices.

Schedule notes (v2):
- Weight/x/ctx DMAs are batched into a handful of large multi-dim
  descriptors (sync queue for weights + ctx transposes, gpsimd for x/ctx
  loads and stores) so queue-issue time stops being a bottleneck.
- x layernorm transposes run on the (otherwise idle) PE via identity
  matmuls; LN normalize runs on ACT (Identity with per-row scale+bias)
  so DVE only does stats + copies.
- sim matmuls for the two heads of a pair are issued back-to-back into
  disjoint PE row groups (K=64 each) so they execute concurrently.
- softmax probs are written as fp8e4 directly by the ACT exp; AV runs as
  fp8 DoubleRow over j-tile pairs (vo pair tiles [128,2,80], ones column
  at col 64 gives the softmax denominator for free).
- FF1 up-proj matmuls are emitted in small "steps" interleaved into the
  ctx and attention phases to fill PE gaps; SwiGLU (silu*a) is deferred
  into phase E where ACT is free, h overwrites gT in place.
- Phase E streams Wo + FF2 with N=1024 rhs into 4 row-tile PSUM banks and
  stores results straight from PSUM to DRAM.
"""

import sys

if "/opt/trn_rl_repo" not in sys.path:
    sys.path.insert(0, "/opt/trn_rl_repo")

import numpy as np
import ml_dtypes

import concourse.bass as bass
import concourse.tile as tile
from concourse import mybir, bacc
from concourse.masks import make_identity

F32 = mybir.dt.float32
BF16 = mybir.dt.bfloat16
FP8 = mybir.dt.float8e4

B, N, J = 2, 2048, 2048
DIM, HEADS, DH = 1024, 16, 64
INNER = HEADS * DH
FF = 4 * DIM
EPS = 1e-5
N_CORES = 8
R = B * N // N_CORES  # 512 rows per core
KT = DIM // 128  # 8 contraction tiles over dim
KP = KT // 2  # 4 fp8 contraction pairs
RT = R // 128  # 4 row tiles
CT = J // 128  # 16 context row tiles
FT = FF // 128  # 32 ff tiles
QSC = 256.0  # host pre-scale on Wq for fp8
VOW = 80  # vo pair tile free width (65 used; 80 for DR step%16==0)


def _ap3(d, n0, n1, n2, off=0):
    """3D view [n0 part, n1, n2] over a 2D dram tensor d with row length n2."""
    return bass.AP(
        tensor=d.tensor,
        offset=d.offset + off,
        ap=[[n2, n0], [n0 * n2, n1], [1, n2]],
    )


def build_kernel(x_bias_nonzero: bool, c_bias_nonzero: bool):
    nc = bacc.Bacc(
        "TRN2", target_bir_lowering=False, debug=False, num_devices=N_CORES
    )
    d_x = nc.dram_tensor("x", [R, DIM], F32, kind="ExternalInput").ap()
    d_ctx = nc.dram_tensor("ctx", [J, DIM], F32, kind="ExternalInput").ap()
    d_wq = nc.dram_tensor("wq", [DIM, INNER], FP8, kind="ExternalInput").ap()
    d_wkv = nc.dram_tensor("wkv", [DIM, 2 * DH], BF16, kind="ExternalInput").ap()
    d_wo = nc.dram_tensor("wo", [INNER, DIM], BF16, kind="ExternalInput").ap()
    d_wff1 = nc.dram_tensor("wff1", [DIM, 2 * FF], BF16, kind="ExternalInput").ap()
    d_wff2 = nc.dram_tensor("wff2", [FF, DIM], BF16, kind="ExternalInput").ap()
    d_xb = (
        nc.dram_tensor("xb", [1, DIM], F32, kind="ExternalInput").ap()
        if x_bias_nonzero
        else None
    )
    d_cb = (
        nc.dram_tensor("cb", [1, DIM], F32, kind="ExternalInput").ap()
        if c_bias_nonzero
        else None
    )
    d_out = nc.dram_tensor("out", [R, DIM], F32, kind="ExternalOutput").ap()

    with tile.TileContext(nc) as tc:
        with (
            tc.tile_pool(name="consts", bufs=1) as consts,
            tc.tile_pool(name="persist", bufs=1) as persist,
            tc.tile_pool(name="wo", bufs=1) as wo_pool,
        ):
            ident = consts.tile([128, 128], BF16)
            make_identity(nc, ident)
            eps_tile = consts.tile([128, 1], F32, tag="eps")
            nc.vector.memset(eps_tile[:], EPS)

            xb_tile = cb_tile = None
            if d_xb is not None:
                xb_tile = consts.tile([128, DIM], F32, tag="xb")
                nc.gpsimd.dma_start(
                    xb_tile[:],
                    bass.AP(
                        tensor=d_xb.tensor, offset=d_xb.offset,
                        ap=[[0, 128]] + d_xb.ap[1:],
                    ),
                )
            if d_cb is not None:
                cb_tile = consts.tile([128, DIM], F32, tag="cb")
                nc.gpsimd.dma_start(
                    cb_tile[:],
                    bass.AP(
                        tensor=d_cb.tensor, offset=d_cb.offset,
                        ap=[[0, 128]] + d_cb.ap[1:],
                    ),
                )

            # ---- persistent SBUF tensors ----
            xnT = persist.tile([128, KT, R], BF16, tag="xnT")
            kT = persist.tile([128, J], BF16, tag="kT")
            vo8 = [
                persist.tile([128, 2, VOW], FP8, tag=f"vo{t}", name=f"vo{t}")
                for t in range(CT // 2)
            ]
            qT = [
                persist.tile([128, R], BF16, tag=f"qT{h}", name=f"qT{h}")
                for h in range(HEADS // 2)
            ]
            aoT = [
                persist.tile([128, R], BF16, tag=f"aoT{k}", name=f"aoT{k}")
                for k in range(KT)
            ]
            aT = [
                persist.tile([128, R], BF16, tag=f"aT{f}", name=f"aT{f}")
                for f in range(FT)
            ]
            gT = [
                persist.tile([128, R], BF16, tag=f"gT{f}", name=f"gT{f}")
                for f in range(FT)
            ]
            wkv_sb = persist.tile([128, KT, 2 * DH], BF16, tag="wkv")
            wo_sb = wo_pool.tile([128, KT, DIM], BF16, tag="wosb")

            # ones column for the AV denominator
            for t in range(CT // 2):
                nc.vector.memset(vo8[t][:, :, DH : DH + 1], 1.0)

            with tc.tile_pool(name="wff1", bufs=2) as wff1_pool:
                # ---- FF1 unit generator (paced into phases C and D) ----
                w1_tiles = {}

                def w1_dma(g):
                    t = wff1_pool.tile([128, 2 * KT, 512], BF16, tag="w1g")
                    # halves in separate kk ranges: kk = half*8 + k
                    for half in range(2):
                        nc.gpsimd.dma_start(
                            t[:, half * KT : (half + 1) * KT, :],
                            bass.AP(
                                tensor=d_wff1.tensor,
                                offset=d_wff1.offset + half * FF + g * 512,
                                ap=[[2 * FF, 128], [128 * 2 * FF, KT], [1, 512]],
                            ),
                        )
                    w1_tiles[g] = t

                def ff1_steps(fi):
                    g = fi // 4
                    fl = fi % 4
                    if fl == 0 and g + 2 < FT // 4:
                        w1_dma(g + 2)
                    w1 = w1_tiles[g]
                    a_ps = psum_f.tile([128, R], F32, tag="ffa")
                    g_ps = psum_f.tile([128, R], F32, tag="ffg")

                    def mk_chain(ps, kk0, k0):
                        def emit():
                            for k in range(k0, k0 + 4):
                                nc.tensor.matmul(
                                    ps[:],
                                    w1[:, kk0 + k, fl * 128 : (fl + 1) * 128],
                                    xnT[:, k, :],
                                    start=(k == 0),
                                    stop=(k == KT - 1),
                                )
                        return emit

                    def finish():
                        for k in range(4, 8):
                            nc.tensor.matmul(
                                g_ps[:],
                                w1[:, KT + k, fl * 128 : (fl + 1) * 128],
                                xnT[:, k, :],
                                start=False,
                                stop=(k == KT - 1),
                            )
                        nc.vector.tensor_copy(aT[fi][:], a_ps[:])
                        nc.vector.tensor_copy(gT[fi][:], g_ps[:])

                    return [
                        mk_chain(a_ps, 0, 0),
                        mk_chain(a_ps, 0, 4),
                        mk_chain(g_ps, KT, 0),
                        finish,
                    ]

                ff_queue = []
                ff_next = [0]

                def ff_step(n=1):
                    for _ in range(n):
                        if not ff_queue and ff_next[0] < FT:
                            ff_queue.extend(ff1_steps(ff_next[0]))
                            ff_next[0] += 1
                        if ff_queue:
                            ff_queue.pop(0)()

                with (
                    tc.tile_pool(name="ln", bufs=5) as ln_pool,
                    tc.tile_pool(name="stats", bufs=10) as stats_pool,
                    tc.tile_pool(name="psF", bufs=1, space="PSUM") as psum_f,
                ):
                    # ---- prologue DMAs (issue order matters per queue) ----
                    # gpsimd: x then ctx chunk 0
                    # sync: wq8, wkv, wff1 g0/g1, wo
                    def ln_stats(src2d, bias_tile):
                        """LN stats for one [128, DIM] f32 tile; returns
                        (rstd[128,1], mbias[128,1]) for ACT normalize."""
                        stats = stats_pool.tile(
                            [128, 2, nc.vector.BN_STATS_DIM], F32, tag="st"
                        )
                        nc.vector.bn_stats(stats[:, 0, :], src2d[:, 0:512])
                        nc.vector.bn_stats(stats[:, 1, :], src2d[:, 512:1024])
                        mv = stats_pool.tile([128, nc.vector.BN_AGGR_DIM], F32, tag="mv")
                        nc.vector.bn_aggr(mv[:], stats[:])
                        rstd = stats_pool.tile([128, 1], F32, tag="rs")
                        nc.scalar.activation(
                            rstd[:], mv[:, 1:2],
                            mybir.ActivationFunctionType.Sqrt, bias=eps_tile[:],
                        )
                        nc.vector.reciprocal(rstd[:], rstd[:])
                        mb = stats_pool.tile([128, 1], F32, tag="mb")
                        nc.vector.tensor_scalar(
                            out=mb[:], in0=mv[:, 0:1],
                            scalar1=rstd[:], scalar2=-1.0,
                            op0=mybir.AluOpType.mult, op1=mybir.AluOpType.mult,
                        )
                        return rstd, mb

                    def ln_normalize(dst, src2d, rstd, mb, bias_tile):
                        # dst = src*rstd - mu*rstd  (ACT Identity, per-row)
                        nc.scalar.activation(
                            dst, src2d,
                            mybir.ActivationFunctionType.Identity,
                            bias=mb[:], scale=rstd[:],
                        )
                        if bias_tile is not None:
                            nc.vector.tensor_add(dst, dst, bias_tile[:])

                    cnload = tc.alloc_tile_pool(name="cnload", bufs=6)
                    cn_tiles = {}

                    def cn_dma(j):
                        t = cnload.tile([128, DIM], F32, tag="cn", name=f"cn{j}")
                        nc.gpsimd.dma_start(t[:], d_ctx[j * 128 : (j + 1) * 128, :])
                        cn_tiles[j] = t

                    with (
                        tc.tile_pool(name="xload", bufs=2) as xload,
                        tc.tile_pool(name="wq", bufs=1) as wq_pool,
                        tc.tile_pool(name="psTR", bufs=2, space="PSUM") as psum_tr,
                        tc.tile_pool(name="psQ", bufs=2, space="PSUM") as psum_q,
                    ):
                        x_ts = []
                        for t in range(RT):
                            x_t = xload.tile([128, DIM], F32, tag="xa", name=f"x{t}")
                            nc.gpsimd.dma_start(
                                x_t[:], d_x[t * 128 : (t + 1) * 128, :]
                            )
                            x_ts.append(x_t)
                        for j in range(6):
                            cn_dma(j)
                        wq_sb = wq_pool.tile([128, KT, INNER], FP8, tag="wq")
                        nc.sync.dma_start(wq_sb[:], _ap3(d_wq, 128, KT, INNER))

                        # ---- Phase A: x layernorm + PE transposes ----
                        for t in range(RT):
                            rstd, mb = ln_stats(x_ts[t][:, :], xb_tile)
                            xn_t = ln_pool.tile([128, DIM], BF16, tag="ln_xn")
                            ln_normalize(xn_t[:], x_ts[t][:, :], rstd, mb, xb_tile)
                            for half in range(2):
                                tr = psum_tr.tile([128, 4, 128], BF16, tag="tr")
                                for q in range(4):
                                    k = half * 4 + q
                                    nc.tensor.transpose(
                                        tr[:, q, :],
                                        xn_t[:, k * 128 : (k + 1) * 128],
                                        ident[:],
                                    )
                                nc.vector.tensor_copy(
                                    xnT[:, half * 4 : (half + 1) * 4,
                                        t * 128 : (t + 1) * 128],
                                    tr[:],
                                )

                        # weight prefetches behind x/ctx loads on gpsimd
                        w1_dma(0)
                        w1_dma(1)
                        nc.gpsimd.dma_start(
                            wkv_sb[:], _ap3(d_wkv, 128, KT, 2 * DH)
                        )
                        nc.gpsimd.dma_start(wo_sb[:], _ap3(d_wo, 128, KT, DIM))

                        # ---- Phase B: Q projection (bf16) ----
                        for hp in range(HEADS // 2):
                            q_ps = psum_q.tile([128, R], F32, tag="q")
                            for k in range(KT):
                                nc.tensor.matmul(
                                    q_ps[:],
                                    wq_sb[:, k, hp * 128 : (hp + 1) * 128],
                                    xnT[:, k, :],
                                    start=(k == 0),
                                    stop=(k == KT - 1),
                                )
                            nc.vector.tensor_scalar(
                                out=qT[hp][:], in0=q_ps[:],
                                scalar1=1.0 / QSC, scalar2=None,
                                op0=mybir.AluOpType.mult,
                            )
                            ff_step(3)

                    # ---- Phase C: ctx layernorm + KV projection ----
                    with (
                        tc.tile_pool(name="cnT", bufs=2) as cnT_pool,
                        tc.tile_pool(name="vstage", bufs=2) as vstage,
                        tc.tile_pool(name="psKV", bufs=2, space="PSUM") as psum_kv,
                        tc.tile_pool(name="psVT", bufs=2, space="PSUM") as psum_vt,
                    ):
                        for c in range(J // 512):
                            cnT = cnT_pool.tile([128, KT, 512], BF16, tag="cnT")
                            for t4 in range(4):
                                j = 4 * c + t4
                                if j + 6 < CT:
                                    cn_dma(j + 6)
                                src = cn_tiles[j][:, :]
                                rstd, mb = ln_stats(src, cb_tile)
                                cn_t = ln_pool.tile([128, DIM], BF16, tag="ln_cn")
                                ln_normalize(cn_t[:], src, rstd, mb, cb_tile)
                                nc.sync.dma_start_transpose(
                                    cnT[:, :, t4 * 128 : (t4 + 1) * 128], cn_t[:]
                                )
                                ff_step(1)
                            kv_ps = psum_kv.tile([128, 512], F32, tag="kv")
                            for k in range(KT):
                                nc.tensor.matmul(
                                    kv_ps[:],
                                    wkv_sb[:, k, :],
                                    cnT[:, k, :],
                                    start=(k == 0),
                                    stop=(k == KT - 1),
                                )
                            nc.vector.tensor_copy(
                                kT[0:DH, c * 512 : (c + 1) * 512], kv_ps[0:DH, :]
                            )
                            nc.gpsimd.dma_start(
                                kT[DH:128, c * 512 : (c + 1) * 512],
                                kT[0:DH, c * 512 : (c + 1) * 512],
                            )
                            vT_sb = vstage.tile([128, 512], BF16, tag="vT")
                            nc.vector.tensor_copy(vT_sb[DH:128, :], kv_ps[DH:128, :])
                            for j4 in range(4):
                                jc = c * 4 + j4
                                vps = psum_vt.tile([128, DH], BF16, tag="vtr")
                                nc.tensor.transpose(
                                    vps[:],
                                    vT_sb[DH:128, j4 * 128 : (j4 + 1) * 128],
                                    ident[DH:128, DH:128],
                                )
                                nc.vector.tensor_copy(
                                    vo8[jc // 2][:, jc % 2, 0:DH], vps[:]
                                )
                            ff_step(2)

                    cnload.release()

                    # ---- Phase D: attention (sim row-packed, AV fp8 DR) ----
                    with (
                        tc.tile_pool(name="attn", bufs=16) as a_pool,
                        tc.tile_pool(name="smx", bufs=3) as smx_pool,
                        tc.tile_pool(name="psS", bufs=2, space="PSUM") as psum_s,
                        tc.tile_pool(name="psAV", bufs=2, space="PSUM") as psum_av,
                    ):
                        for hp in range(HEADS // 2):
                            a_sbs = [[], []]
                            for p in range(CT // 2):
                                s0 = psum_s.tile([128, 2, R], F32, tag="s")
                                s1 = psum_s.tile([128, 2, R], F32, tag="s")
                                for half in range(2):
                                    jc = 2 * p + half
                                    nc.tensor.matmul(
                                        s0[:, half, :],
                                        kT[0:DH, jc * 128 : (jc + 1) * 128],
                                        qT[hp][0:DH, :],
                                        start=True,
                                        stop=True,
                                    )
                                    nc.tensor.matmul(
                                        s1[:, half, :],
                                        kT[DH:128, jc * 128 : (jc + 1) * 128],
                                        qT[hp][DH:128, :],
                                        start=True,
                                        stop=True,
                                    )
                                for h2, s in ((0, s0), (1, s1)):
                                    a_t = a_pool.tile([128, 2, R], FP8, tag="a")
                                    nc.scalar.activation(
                                        a_t[:], s[:],
                                        mybir.ActivationFunctionType.Exp,
                                    )
                                    a_sbs[h2].append(a_t)
                                ff_step(2)
                            av = [
                                psum_av.tile([DH + 1, R], F32, tag="av", name=f"av{h2}")
                                for h2 in range(2)
                            ]
                            for p in range(CT // 2):
                                for h2 in range(2):
                                    nc.tensor.matmul(
                                        av[h2][:],
                                        vo8[p][:, :, 0 : DH + 1],
                                        a_sbs[h2][p][:],
                                        start=(p == 0),
                                        stop=(p == CT // 2 - 1),
                                        perf_mode=mybir.MatmulPerfMode.DoubleRow,
                                    )
                                if p in (2, 5):
                                    ff_step(1)
                            for h2 in range(2):
                                rec = smx_pool.tile([DH + 1, R], F32, tag="rec")
                                nc.vector.tensor_copy(
                                    rec[DH : DH + 1, :], av[h2][DH : DH + 1, :]
                                )
                                r0 = smx_pool.tile([1, R], F32, tag="r0")
                                nc.gpsimd.dma_start(r0[:], rec[DH : DH + 1, :])
                                rbc = smx_pool.tile([DH, R], F32, tag="rbc")
                                nc.gpsimd.partition_broadcast(rbc[:], r0[:])
                                nc.vector.reciprocal_approx_fast(rbc[:], rbc[:])
                                if h2 == 0:
                                    nc.vector.tensor_mul(
                                        aoT[hp][0:DH, :], av[h2][0:DH, :], rbc[:]
                                    )
                                else:
                                    tmp = smx_pool.tile([DH, R], BF16, tag="tmp")
                                    nc.vector.tensor_mul(
                                        tmp[:], av[h2][0:DH, :], rbc[:]
                                    )
                                    nc.gpsimd.dma_start(aoT[hp][DH:128, :], tmp[:])
                            ff_step(2)
                        while ff_queue or ff_next[0] < FT:
                            ff_step(1)

            # ---- Phase E: Wo + SwiGLU + FF2, N=1024 rhs, 4 row-tile psums ----
            with (
                tc.tile_pool(name="wff2", bufs=2) as wff2_pool,
                tc.tile_pool(name="hstage", bufs=3) as hstage,
                tc.tile_pool(name="psO", bufs=1, space="PSUM") as psum_o,
            ):
                w2_tiles = {}

                def w2_dma(g):
                    t = wff2_pool.tile([128, 8, DIM], BF16, tag="w2")
                    nc.sync.dma_start(
                        t[:], _ap3(d_wff2, 128, 8, DIM, off=g * 8 * 128 * DIM)
                    )
                    w2_tiles[g] = t

                h_next = [0]

                def h_step(n=1):
                    for _ in range(n):
                        fi = h_next[0]
                        if fi >= FT:
                            return
                        h_next[0] += 1
                        sil = hstage.tile([128, R], F32, tag="sil")
                        nc.scalar.activation(
                            sil[:], gT[fi][:],
                            mybir.ActivationFunctionType.Silu,
                        )
                        nc.vector.tensor_mul(gT[fi][:], aT[fi][:], sil[:])

                w2_dma(0)
                for ch in range(2):
                    o_ps = [
                        psum_o.tile(
                            [128, 512], F32, tag=f"o{ch}{rs}", name=f"o{ch}{rs}"
                        )
                        for rs in range(RT)
                    ]
                    for k in range(KT):
                        for rs in range(RT):
                            nc.tensor.matmul(
                                o_ps[rs][:],
                                aoT[k][:, rs * 128 : (rs + 1) * 128],
                                wo_sb[:, k, ch * 512 : (ch + 1) * 512],
                                start=(k == 0),
                                stop=False,
                            )
                        if ch == 0 and k == 1:
                            w2_dma(1)
                        if ch == 0 and k >= 4:
                            h_step(1)
                    for g in range(4):
                        for f8 in range(8):
                            fi = g * 8 + f8
                            if ch == 0:
                                h_step(1)
                            for rs in range(RT):
                                nc.tensor.matmul(
                                    o_ps[rs][:],
                                    gT[fi][:, rs * 128 : (rs + 1) * 128],
                                    w2_tiles[g][:, f8, ch * 512 : (ch + 1) * 512],
                                    start=False,
                                    stop=(fi == FT - 1),
                                )
                        if ch == 0 and g + 2 < 4:
                            w2_dma(g + 2)
                    for rs in range(RT):
                        o_sb = hstage.tile([128, 512], F32, tag="ost")
                        nc.scalar.activation(
                            o_sb[:], o_ps[rs][:],
                            mybir.ActivationFunctionType.Copy,
                        )
                        nc.gpsimd.dma_start(
                            d_out[
                                rs * 128 : (rs + 1) * 128,
                                ch * 512 : (ch + 1) * 512,
                            ],
                            o_sb[:],
                        )

    nc.compile()
    return nc


_NC_CACHE = {}


def _get_nc(x_bias_nonzero, c_bias_nonzero):
    key = (x_bias_nonzero, c_bias_nonzero)
    if key not in _NC_CACHE:
        _NC_CACHE[key] = build_kernel(*key)
    return _NC_CACHE[key]


def make_in_maps(x, context, norm_g, norm_b, cnorm_g, cnorm_b, Wq, Wkv, Wo, Wff1, Wff2):
    x = np.asarray(x, np.float32)
    context = np.asarray(context, np.float32)
    norm_g = np.asarray(norm_g, np.float32)
    norm_b = np.asarray(norm_b, np.float32)
    cnorm_g = np.asarray(cnorm_g, np.float32)
    cnorm_b = np.asarray(cnorm_b, np.float32)
    scale = DH ** -0.5
    bf = ml_dtypes.bfloat16
    f8 = ml_dtypes.float8_e4m3
    wq = np.ascontiguousarray(
        np.clip(
            norm_g[:, None] * np.asarray(Wq, np.float32) * scale * QSC, -240, 240
        )
    ).astype(f8)
    wkv = np.ascontiguousarray(cnorm_g[:, None] * np.asarray(Wkv, np.float32)).astype(bf)
    wo = np.ascontiguousarray(np.asarray(Wo, np.float32)).astype(bf)
    wff1 = np.ascontiguousarray(norm_g[:, None] * np.asarray(Wff1, np.float32)).astype(bf)
    wff2 = np.ascontiguousarray(np.asarray(Wff2, np.float32)).astype(bf)
    x_bias = bool(np.any(norm_b != 0.0))
    c_bias = bool(np.any(cnorm_b != 0.0))
    in_maps = []
    for c in range(N_CORES):
        b = c // (N_CORES // B)
        r0 = (c % (N_CORES // B)) * R
        m = {
            "x": np.ascontiguousarray(x[b, r0 : r0 + R, :]),
            "ctx": np.ascontiguousarray(context[b]),
            "wq": wq,
            "wkv": wkv,
            "wo": wo,
            "wff1": wff1,
            "wff2": wff2,
        }
        if x_bias:
            m["xb"] = norm_b.reshape(1, DIM).copy()
        if c_bias:
            m["cb"] = cnorm_b.reshape(1, DIM).copy()
        in_maps.append(m)
    return in_maps, x_bias, c_bias


def gather_output(results):
    out = np.empty((B, N, DIM), np.float32)
    for c in range(N_CORES):
        b = c // (N_CORES // B)
        r0 = (c % (N_CORES // B)) * R
        out[b, r0 : r0 + R, :] = results[c]["out"]
    return out


def kernel(**inputs):
    from concourse.bass_utils import run_bass_kernel_spmd

    in_maps, x_bias, c_bias = make_in_maps(**inputs)
    nc = _get_nc(x_bias, c_bias)
    res = run_bass_kernel_spmd(nc, in_maps, list(range(N_CORES)))
    return gather_output(res.results)
